# revision 1
# baseline (speedup 1.0000x reference)
"""CrossAttentionHead kernel for 8 trn2 NeuronCores.

Sharding: core i handles batch b = i//2, query rows half = i%2 (2048 rows).
Each core gets x_shard [2048,1024], full z[b] [4096,1024], Wq/Wk/Wv [128,1024]
and produces out [2048,128]. SPMD: identical program, per-core input data.

Per-core pipeline (all on-chip after initial loads):
  1. PE-transpose Wq/Wk/Wv -> WT [e-chunk,h] layout.
  2. PE-transpose x,z tiles -> xT/zT [e,seq]; project qT=[h,lq], kT=[h,lk]
     (accumulate over 8 e-chunks), v natural [lk,h] (bf16 path optional).
  3. Per 128-row query tile: scores s = qT_tile.T @ kT in 8 chunks of 512
     (PSUM); per-chunk row-max (DVE) -> exp((s-m_chunk)*scale) via ScalarE
     activation with per-partition bias + accum_out row-sums; deferred
     correction exp(scale*(m_chunk-m_row)) multiplied into w chunks;
     PE-transpose w -> wT; AV: out_psum += wT_i.T @ v_i over 32 lk chunks;
     normalize by reciprocal row-sum during PSUM->SBUF eviction; DMA out.
"""
import sys
sys.path.insert(0, "/opt/trn_rl_repo")

import math
import numpy as np

import concourse.bass as bass
import concourse.mybir as mybir
import concourse.tile as tile
from concourse import bacc
from concourse.bass_utils import run_bass_kernel_spmd
from concourse.masks import make_identity

F32 = mybir.dt.float32
F32R = mybir.dt.float32r
BF16 = mybir.dt.bfloat16
FP16 = mybir.dt.float16

B, LQ, LKV, E, H = 4, 4096, 4096, 1024, 128
LQS = LQ // 2          # 2048 query rows per core
SCALE = math.sqrt(float(H))
N_CORES = 8

# --- tunables (defaults chosen via cost-model sweeps) ---------------------
import os
def _knob(name, default):
    return int(os.environ.get(name, default))
NC_E = E // 128        # 8 e-chunks
NT_Q = LQS // 128      # 16 query tiles per core
NG_Q = LQS // 512      # 4 query groups (512) per core
NG_K = LKV // 512      # 8 kv groups
NC_K = LKV // 128      # 32 kv chunks


def build_bass():
    nc = bacc.Bacc("TRN2", target_bir_lowering=False, debug=True)
    x_hi = nc.declare_dram_parameter("x_hi", [LQS, E], BF16, isOutput=False)
    x_lo = nc.declare_dram_parameter("x_lo", [LQS, E], BF16, isOutput=False)
    z_hi = nc.declare_dram_parameter("z_hi", [LKV, E], BF16, isOutput=False)
    z_lo = nc.declare_dram_parameter("z_lo", [LKV, E], BF16, isOutput=False)
    Wq = nc.declare_dram_parameter("Wq", [H, E], F32, isOutput=False)
    Wk = nc.declare_dram_parameter("Wk", [H, E], F32, isOutput=False)
    Wv = nc.declare_dram_parameter("Wv", [H, E], F32, isOutput=False)
    out = nc.declare_dram_parameter("out", [LQS, H], F32, isOutput=True)

    wdt = FP16

    with tile.TileContext(nc) as tc:
        with tc.tile_pool(name="consts", bufs=1) as consts, \
             tc.tile_pool(name="persist", bufs=1) as persist:
            wnats = []
            for W_in in (Wq, Wk, Wv):
                wnat = consts.tile([128, E], F32, tag=f"wnat{len(wnats)}")
                nc.gpsimd.dma_start(wnat[:], W_in[:])
                wnats.append(wnat)
            ident = consts.tile([128, 128], F32, tag="ident")
            make_identity(nc, ident[:])
            identw = consts.tile([128, 128], wdt, tag="identw")
            make_identity(nc, identw[:])

            qThi = persist.tile([128, LQS], BF16, tag="qThi")    # [h, lq]
            qTlo = persist.tile([128, LQS], BF16, tag="qTlo")
            kThi = persist.tile([128, LKV], BF16, tag="kThi")    # [h, lk]
            kTlo = persist.tile([128, LKV], BF16, tag="kTlo")
            v = persist.tile([128, NC_K * 128], wdt, tag="v")   # [lk128, 32*h]
            wqThi = persist.tile([128, E], BF16, tag="wqThi")    # [e128, 8*h]
            wqTlo = persist.tile([128, E], BF16, tag="wqTlo")
            wkThi = persist.tile([128, E], BF16, tag="wkThi")
            wkTlo = persist.tile([128, E], BF16, tag="wkTlo")
            wvT16 = persist.tile([128, E], FP16, tag="wvT16")

            # ---- phases 1+2: W/x/z transposes + projections (shared pools) ----
            with tc.tile_pool(name="ph2nat", bufs=_knob("PH2NAT", 6)) as ph2nat, \
                 tc.tile_pool(name="ph2t", bufs=_knob("PH2T", 2)) as ph2t, \
                 tc.tile_pool(name="ph2tb", bufs=2) as ph2tb, \
                 tc.tile_pool(name="ph2ps", bufs=_knob("PH2PS", 4), space="PSUM") as ph2ps, \
                 tc.tile_pool(name="ph2acc", bufs=_knob("PH2ACC", 2), space="PSUM") as ph2acc:
                for wnat, wT_hi, wT_lo in ((wnats[0], wqThi, wqTlo),
                                           (wnats[1], wkThi, wkTlo),
                                           (wnats[2], wvT16, None)):
                    for q4 in range(2):
                        pt = ph2ps.tile([128, 512], F32, tag="pt")
                        for s4 in range(4):
                            c = q4 * 4 + s4
                            nc.tensor.transpose(
                                pt[:, s4 * 128:(s4 + 1) * 128],
                                wnat[:, c * 128:(c + 1) * 128], ident[:])
                        cs = slice(q4 * 512, (q4 + 1) * 512)
                        nc.scalar.copy(wT_hi[:, cs], pt[:])
                        if wT_lo is not None:
                            nc.vector.tensor_tensor(
                                wT_lo[:, cs], pt[:], wT_hi[:, cs],
                                op=mybir.AluOpType.subtract)

                def load_transpose_group(src_hi, src_lo, g):
                    """Rows [g*512,+512) of hi/lo -> transposed [e128,chunk,512]
                    via xbar DMA transpose (2-byte dtype), no PE involvement."""
                    sThi = ph2t.tile([128, NC_E, 512], BF16, tag="sThi")
                    sTlo = ph2t.tile([128, NC_E, 512], BF16, tag="sTlo")
                    rows = slice(g * 512, (g + 1) * 512)
                    for c in range(NC_E):
                        cols = slice(c * 128, (c + 1) * 128)
                        nc.sync.dma_start_transpose(
                            sThi[:, c, :], src_hi[rows, cols])
                        nc.sync.dma_start_transpose(
                            sTlo[:, c, :], src_lo[rows, cols])
                    return sThi, sTlo

                for g in range(NG_K):
                    zThi, zTlo = load_transpose_group(z_hi, z_lo, g)
                    kps = ph2acc.tile([128, 512], F32, tag="acc")
                    for c in range(NC_E):
                        cs = slice(c * 128, (c + 1) * 128)
                        nc.tensor.matmul(kps[:], wkThi[:, cs], zThi[:, c, :],
                                         start=(c == 0), stop=False)
                        nc.tensor.matmul(kps[:], wkThi[:, cs], zTlo[:, c, :],
                                         start=False, stop=False)
                        nc.tensor.matmul(kps[:], wkTlo[:, cs], zThi[:, c, :],
                                         start=False, stop=(c == NC_E - 1))
                    khi = kThi[:, g * 512:(g + 1) * 512]
                    nc.scalar.copy(khi, kps[:])
                    nc.vector.tensor_tensor(
                        kTlo[:, g * 512:(g + 1) * 512], kps[:], khi,
                        op=mybir.AluOpType.subtract)

                    zf16 = ph2tb.tile([128, NC_E, 512], FP16, tag="zf16")
                    nc.gpsimd.tensor_tensor(zf16[:], zThi[:], zTlo[:],
                                            op=mybir.AluOpType.add)
                    # v natural [lk,h]: per 128-row subtile accumulate e-chunks
                    for s in range(4):
                        vps = ph2acc.tile([128, 128], F32, tag="vacc")
                        for c in range(NC_E):
                            nc.tensor.matmul(
                                vps[:],
                                zf16[:, c, s * 128:(s + 1) * 128],
                                wvT16[:, c * 128:(c + 1) * 128],
                                start=(c == 0), stop=(c == NC_E - 1))
                        i = g * 4 + s
                        nc.vector.tensor_copy(
                            v[:, i * 128:(i + 1) * 128], vps[:])

                for g in range(NG_Q):
                    xThi, xTlo = load_transpose_group(x_hi, x_lo, g)
                    qps = ph2acc.tile([128, 512], F32, tag="acc")
                    for c in range(NC_E):
                        cs = slice(c * 128, (c + 1) * 128)
                        nc.tensor.matmul(qps[:], wqThi[:, cs], xThi[:, c, :],
                                         start=(c == 0), stop=False)
                        nc.tensor.matmul(qps[:], wqThi[:, cs], xTlo[:, c, :],
                                         start=False, stop=False)
                        nc.tensor.matmul(qps[:], wqTlo[:, cs], xThi[:, c, :],
                                         start=False, stop=(c == NC_E - 1))
                    hi = qThi[:, g * 512:(g + 1) * 512]
                    nc.scalar.copy(hi, qps[:])
                    nc.vector.tensor_tensor(
                        qTlo[:, g * 512:(g + 1) * 512], qps[:], hi,
                        op=mybir.AluOpType.subtract)

            # ---- phase 3: attention per 128-row query tile ----
            nt_q = NT_Q if _knob("PHASES", 3) >= 3 else 0
            with tc.tile_pool(name="ph3w", bufs=_knob("PH3W", 2)) as ph3w, \
                 tc.tile_pool(name="ph3wt", bufs=_knob("PH3WT", 2)) as ph3wt, \
                 tc.tile_pool(name="ph3sm", bufs=2) as ph3sm, \
                 tc.tile_pool(name="ph3o", bufs=2) as ph3o, \
                 tc.tile_pool(name="ph3ps", bufs=_knob("PH3PS", 4), space="PSUM") as ph3ps, \
                 tc.tile_pool(name="ph3pt", bufs=_knob("PH3PT", 2), space="PSUM") as ph3pt, \
                 tc.tile_pool(name="ph3po", bufs=_knob("PH3PO", 2), space="PSUM") as ph3po:
                for t in range(nt_q):
                    qThit = qThi[:, t * 128:(t + 1) * 128]
                    qTlot = qTlo[:, t * 128:(t + 1) * 128]
                    w = ph3w.tile([128, LKV], wdt, tag="w")
                    mloc = ph3sm.tile([128, 8], F32, tag="mloc")
                    negm = ph3sm.tile([128, 8], F32, tag="negm")
                    lparts = ph3sm.tile([128, 8], F32, tag="lparts")
                    for j in range(8):
                        sp = ph3ps.tile([128, 512], F32, tag="sp")
                        kchunk = slice(j * 512, (j + 1) * 512)
                        nc.tensor.matmul(sp[:], qThit, kThi[:, kchunk],
                                         start=True, stop=False)
                        nc.tensor.matmul(sp[:], qThit, kTlo[:, kchunk],
                                         start=False, stop=False)
                        nc.tensor.matmul(sp[:], qTlot, kThi[:, kchunk],
                                         start=False, stop=True)
                        nc.vector.tensor_reduce(
                            mloc[:, j:j + 1], sp[:], axis=mybir.AxisListType.X,
                            op=mybir.AluOpType.max)
                        nc.vector.tensor_scalar_mul(
                            negm[:, j:j + 1], mloc[:, j:j + 1], -SCALE)
                        nc.scalar.activation(
                            w[:, j * 512:(j + 1) * 512], sp[:],
                            mybir.ActivationFunctionType.Exp,
                            bias=negm[:, j:j + 1], scale=SCALE,
                            accum_out=lparts[:, j:j + 1])
                    # global row max and per-chunk corrections
                    m = ph3sm.tile([128, 1], F32, tag="m")
                    nc.vector.tensor_reduce(
                        m[:], mloc[:], axis=mybir.AxisListType.X,
                        op=mybir.AluOpType.max)
                    negmg = ph3sm.tile([128, 1], F32, tag="negmg")
                    nc.vector.tensor_scalar_mul(negmg[:], m[:], -SCALE)
                    f = ph3sm.tile([128, 8], F32, tag="f")
                    nc.scalar.activation(
                        f[:], mloc[:], mybir.ActivationFunctionType.Exp,
                        bias=negmg[:], scale=SCALE)
                    fl = ph3sm.tile([128, 8], F32, tag="fl")
                    nc.vector.tensor_tensor(
                        fl[:], f[:], lparts[:], op=mybir.AluOpType.mult)
                    l = ph3sm.tile([128, 1], F32, tag="l")
                    nc.vector.tensor_reduce(
                        l[:], fl[:], axis=mybir.AxisListType.X,
                        op=mybir.AluOpType.add)
                    linv = ph3sm.tile([128, 1], F32, tag="linv")
                    nc.vector.reciprocal(linv[:], l[:])
                    for j in range(8):
                        nc.gpsimd.tensor_scalar_mul(
                            w[:, j * 512:(j + 1) * 512],
                            w[:, j * 512:(j + 1) * 512], f[:, j:j + 1])
                    # transpose w -> wT, 4 chunks per PSUM bank
                    wTt = ph3wt.tile([128, NC_K * 128], wdt, tag="wTt")
                    for q in range(8):
                        pt = ph3pt.tile([128, 512], wdt, tag="pt")
                        for s in range(4):
                            i = q * 4 + s
                            nc.tensor.transpose(
                                pt[:, s * 128:(s + 1) * 128],
                                w[:, i * 128:(i + 1) * 128], identw[:])
                        eng_scalar = (q % 2 == 0)
                        if eng_scalar:
                            nc.scalar.copy(wTt[:, q * 512:(q + 1) * 512], pt[:])
                        else:
                            nc.vector.tensor_copy(
                                wTt[:, q * 512:(q + 1) * 512], pt[:])
                    # AV accumulate
                    ops = ph3po.tile([128, 128], F32, tag="ops")
                    for i in range(NC_K):
                        nc.tensor.matmul(
                            ops[:], wTt[:, i * 128:(i + 1) * 128],
                            v[:, i * 128:(i + 1) * 128],
                            start=(i == 0), stop=(i == NC_K - 1))
                    osb = ph3o.tile([128, 128], F32, tag="osb")
                    nc.vector.tensor_scalar_mul(osb[:], ops[:], linv[:])
                    nc.sync.dma_start(out[t * 128:(t + 1) * 128, :], osb[:])
    nc.finalize()
    return nc


_NC_CACHE = None
TRACE = False
LAST_EXEC_NS = None
LAST_RESULTS = None


def kernel(x, z, Wq, Wk, Wv):
    global _NC_CACHE, LAST_EXEC_NS, LAST_RESULTS
    if _NC_CACHE is None:
        _NC_CACHE = build_bass()
    nc = _NC_CACHE

    import ml_dtypes
    x = np.asarray(x, dtype=np.float32)
    z = np.asarray(z, dtype=np.float32)
    x_hi = x.astype(ml_dtypes.bfloat16)
    x_lo = (x - x_hi.astype(np.float32)).astype(ml_dtypes.bfloat16)
    z_hi = z.astype(ml_dtypes.bfloat16)
    z_lo = (z - z_hi.astype(np.float32)).astype(ml_dtypes.bfloat16)
    Wq = np.ascontiguousarray(np.asarray(Wq, dtype=np.float32))
    Wk = np.ascontiguousarray(np.asarray(Wk, dtype=np.float32))
    Wv = np.ascontiguousarray(np.asarray(Wv, dtype=np.float32))

    in_maps = []
    for core in range(N_CORES):
        b, half = core // 2, core % 2
        rows = slice(half * LQS, (half + 1) * LQS)
        in_maps.append({
            "x_hi": np.ascontiguousarray(x_hi[b, rows]),
            "x_lo": np.ascontiguousarray(x_lo[b, rows]),
            "z_hi": np.ascontiguousarray(z_hi[b]),
            "z_lo": np.ascontiguousarray(z_lo[b]),
            "Wq": Wq, "Wk": Wk, "Wv": Wv,
        })
    if TRACE:
        import os
        tdir = "/root/problem/trace_out"
        os.makedirs(tdir, exist_ok=True)
        br = run_bass_kernel_spmd(nc, in_maps, list(range(N_CORES)),
                                  trace=True, tmpdir=tdir)
        LAST_EXEC_NS = br.exec_time_ns
        LAST_RESULTS = br
        res = br.results
    else:
        res = run_bass_kernel_spmd(nc, in_maps, list(range(N_CORES))).results
    outp = np.empty((B, LQ, H), dtype=np.float32)
    for core in range(N_CORES):
        b, half = core // 2, core % 2
        outp[b, half * LQS:(half + 1) * LQS] = res[core]["out"]
    return outp



# revision 8
# speedup vs baseline: 6.6118x; 6.6118x over previous
"""CrossAttentionHead kernel for 8 trn2 NeuronCores.

Sharding: core i handles batch b = i//2, query rows half = i%2 (2048 rows).
Host pre-transposes and splits inputs into fp16 "hi" + fp8(e5m2) hi/lo
planes so every matmul runs as one fp16 pass plus one half-rate
DoubleRow fp8 cross pass (error ~2^-15 per operand, ~20x cheaper than
the fp32 3-pass scheme).

Per-core dataflow:
  zh[e,lk] fp16 + z8[e,2,lk] fp8(planes: hi8, lo8), same for x,
  W*h = fp16(32*W.T), W*8 = (lo8, hi8) planes, Wv fp16 only.
  K: kps = Wkh.T@zh + DR(Wk8, z8)   (PSUM = 32*k)
     kT16 = fp16(kps)  [=32k],  k8 = (e5m2(kT16), e5m2(kps - kT16))
  Q: qps = 32q; qT16 = fp16(qps*S/1024) [=S*q/32], q8 = (lo8, hi8)
  scores s = qT16.T@kT16 + DR(q8[lo,hi], k8[hi,lo]) = S*q.k in fp32 PSUM.
  Per 128-row tile, 4 kv-groups of 1024 (2 PSUM banks): group max (DVE,
  negated) -> exp with bias + accum row-sums (ACT) -> deferred f
  corrections -> w scaled (Pool/DVE) -> one SBUF->SBUF xbar DMA
  transpose -> AV fp16 -> out = av/l (ACT scale=1/l) -> DMA out.
"""
import sys
sys.path.insert(0, "/opt/trn_rl_repo")

import math
import os
import numpy as np

import concourse.bass as bass
import concourse.mybir as mybir
import concourse.tile as tile
from concourse import bacc
from concourse.bass_utils import run_bass_kernel_spmd

F32 = mybir.dt.float32
FP16 = mybir.dt.float16
FP8 = mybir.dt.float8e5
DR = mybir.MatmulPerfMode.DoubleRow

B, LQ, LKV, E, H = 4, 4096, 4096, 1024, 128
LQS = LQ // 2          # 2048 query rows per core
SCALE = math.sqrt(float(H))
WS = 32.0              # host pre-scale on Wq/Wk
QS = SCALE / (WS * WS)  # q eviction scale: qT16 = S*q/32
N_CORES = 8

NC_E = E // 128        # 8 e-chunks
NG_Z = LKV // 512      # 8 z load groups
NG_X = LQS // 512      # 4 x load groups
NT_Q = LQS // 128      # 16 query tiles per core
NG_K = 4               # kv groups per tile (1024 cols, 2 PSUM banks)
GK = LKV // NG_K       # 1024


def _knob(name, default):
    return int(os.environ.get(name, default))


def build_bass():
    nc = bacc.Bacc("TRN2", target_bir_lowering=False, debug=True)
    xh = nc.declare_dram_parameter("xh", [E, LQS], FP16, isOutput=False)
    x8 = nc.declare_dram_parameter("x8", [E, 2, LQS], FP8, isOutput=False)
    zh = nc.declare_dram_parameter("zh", [E, LKV], FP16, isOutput=False)
    z8 = nc.declare_dram_parameter("z8", [E, 2, LKV], FP8, isOutput=False)
    Wqh = nc.declare_dram_parameter("Wqh", [E, H], FP16, isOutput=False)
    Wq8 = nc.declare_dram_parameter("Wq8", [E, 2, H], FP8, isOutput=False)
    Wkh = nc.declare_dram_parameter("Wkh", [E, H], FP16, isOutput=False)
    Wk8 = nc.declare_dram_parameter("Wk8", [E, 2, H], FP8, isOutput=False)
    Wvh = nc.declare_dram_parameter("Wvh", [E, H], FP16, isOutput=False)
    out = nc.declare_dram_parameter("out", [LQS, H], F32, isOutput=True)

    n_pe_transp = _knob("PE_TRANSP", 0)
    npool = _knob("NPOOL", 3)

    with tile.TileContext(nc) as tc:
        with tc.tile_pool(name="consts", bufs=1) as consts, \
             tc.tile_pool(name="persist", bufs=1) as persist, \
             tc.tile_pool(name="psA", bufs=_knob("PSA", 3), space="PSUM") as psA, \
             tc.tile_pool(name="psB", bufs=_knob("PSB", 2), space="PSUM") as psB:
            wqh = consts.tile([128, NC_E, H], FP16, tag="wqh")
            wq8 = consts.tile([128, NC_E, 2, H], FP8, tag="wq8")
            wkh = consts.tile([128, NC_E, H], FP16, tag="wkh")
            wk8 = consts.tile([128, NC_E, 2, H], FP8, tag="wk8")
            wvh = consts.tile([128, NC_E, H], FP16, tag="wvh")
            nc.sync.dma_start(wqh[:], Wqh[:])
            nc.sync.dma_start(wq8[:], Wq8[:])
            nc.sync.dma_start(wkh[:], Wkh[:])
            nc.sync.dma_start(wk8[:], Wk8[:])
            nc.sync.dma_start(wvh[:], Wvh[:])
            identw = None
            if n_pe_transp > 0:
                from concourse.masks import make_identity
                identw = consts.tile([128, 128], FP16, tag="identw")
                make_identity(nc, identw[:])

            kT16 = persist.tile([128, LKV], FP16, tag="kT16")   # [h, lk] = 32k
            k8 = persist.tile([128, 2, LKV], FP8, tag="k8")     # (hi8, lo8)
            qT16 = persist.tile([128, LQS], FP16, tag="qT16")   # [h, lq] = S q/32
            q8 = persist.tile([128, 2, LQS], FP8, tag="q8")     # (lo8, hi8)
            v = persist.tile([128, LKV], FP16, tag="v")         # [lk128, 32*h]

            # ---- phase 2: projections ----
            with tc.tile_pool(name="ph2in", bufs=_knob("PH2IN", 2)) as ph2in, \
                 tc.tile_pool(name="ph2in8", bufs=_knob("PH2IN8", 2)) as ph2in8, \
                 tc.tile_pool(name="ph2f", bufs=2) as ph2f:
                for g in range(NG_Z):
                    zg = ph2in.tile([128, NC_E, 512], FP16, tag="zg")
                    zg8 = ph2in8.tile([128, NC_E, 2, 512], FP8, tag="zg8")
                    cols = slice(g * 512, (g + 1) * 512)
                    nc.sync.dma_start(zg[:], zh[:, cols])
                    nc.sync.dma_start(zg8[:], z8[:, :, cols])
                    kpsw = psA.tile([128, 2, 512], F32, tag="sp")
                    kps = kpsw[:, 0, :]
                    for c in range(NC_E):
                        nc.tensor.matmul(kps, wkh[:, c, :], zg[:, c, :],
                                         start=(c == 0), stop=False)
                    for c in range(NC_E):
                        nc.tensor.matmul(kps, wk8[:, c, :, :], zg8[:, c, :, :],
                                         perf_mode=DR, skip_group_check=True,
                                         start=False, stop=(c == NC_E - 1))
                    nc.scalar.copy(kT16[:, cols], kps)
                    nc.gpsimd.tensor_copy(k8[:, 0, cols], kT16[:, cols])
                    nc.vector.tensor_tensor(k8[:, 1, cols], kps, kT16[:, cols],
                                            op=mybir.AluOpType.subtract)
                    vps = psB.tile([128, 4, 128], F32, tag="vps")
                    for s in range(4):
                        for c in range(NC_E):
                            nc.tensor.matmul(vps[:, s, :],
                                             zg[:, c, s * 128:(s + 1) * 128],
                                             wvh[:, c, :],
                                             start=(c == 0), stop=(c == NC_E - 1))
                    nc.scalar.copy(v[:, cols], vps[:])

                for g in range(NG_X):
                    xg = ph2in.tile([128, NC_E, 512], FP16, tag="zg")
                    xg8 = ph2in8.tile([128, NC_E, 2, 512], FP8, tag="zg8")
                    cols = slice(g * 512, (g + 1) * 512)
                    nc.sync.dma_start(xg[:], xh[:, cols])
                    nc.sync.dma_start(xg8[:], x8[:, :, cols])
                    qpsw = psA.tile([128, 2, 512], F32, tag="sp")
                    qps = qpsw[:, 0, :]
                    for c in range(NC_E):
                        nc.tensor.matmul(qps, wqh[:, c, :], xg[:, c, :],
                                         start=(c == 0), stop=False)
                    for c in range(NC_E):
                        nc.tensor.matmul(qps, wq8[:, c, :, :], xg8[:, c, :, :],
                                         perf_mode=DR, skip_group_check=True,
                                         start=False, stop=(c == NC_E - 1))
                    nc.scalar.activation(qT16[:, cols], qps,
                                         mybir.ActivationFunctionType.Copy,
                                         scale=QS)
                    qfull = ph2f.tile([128, 512], F32, tag="qfull")
                    nc.scalar.activation(qfull[:], qps,
                                         mybir.ActivationFunctionType.Copy,
                                         scale=QS)
                    nc.gpsimd.tensor_copy(q8[:, 1, cols], qT16[:, cols])
                    nc.gpsimd.tensor_tensor(q8[:, 0, cols], qfull[:],
                                            qT16[:, cols],
                                            op=mybir.AluOpType.subtract)

            # ---- phase 3: attention per 128-row query tile ----
            with tc.tile_pool(name="ph3w", bufs=_knob("PH3W", 2)) as ph3w, \
                 tc.tile_pool(name="ph3wt", bufs=_knob("PH3WT", 2)) as ph3wt, \
                 tc.tile_pool(name="ph3sm", bufs=_knob("PH3SM", 3)) as ph3sm, \
                 tc.tile_pool(name="ph3o", bufs=2) as ph3o:
                for t in range(NT_Q):
                    tq = slice(t * 128, (t + 1) * 128)
                    w = ph3w.tile([128, LKV], FP16, tag="w")
                    mloc = ph3sm.tile([128, NG_K], F32, tag="mloc")
                    lparts = ph3sm.tile([128, NG_K], F32, tag="lparts")
                    for g in range(NG_K):
                        sp = psA.tile([128, 2, 512], F32, tag="sp")
                        for hh in range(2):
                            kc = slice(g * GK + hh * 512, g * GK + (hh + 1) * 512)
                            nc.tensor.matmul(sp[:, hh, :], qT16[:, tq],
                                             kT16[:, kc], start=True, stop=False)
                            nc.tensor.matmul(sp[:, hh, :], q8[:, :, tq],
                                             k8[:, :, kc],
                                             perf_mode=DR, skip_group_check=True,
                                             start=False, stop=True)
                        nc.vector.tensor_reduce(
                            mloc[:, g:g + 1], sp[:], axis=mybir.AxisListType.XY,
                            op=mybir.AluOpType.max, negate=True)
                        nc.scalar.activation(
                            w[:, g * GK:(g + 1) * GK], sp[:],
                            mybir.ActivationFunctionType.Exp,
                            bias=mloc[:, g:g + 1], scale=1.0,
                            accum_out=lparts[:, g:g + 1])
                    negm = ph3sm.tile([128, 1], F32, tag="negm")
                    nc.vector.tensor_reduce(
                        negm[:], mloc[:], axis=mybir.AxisListType.X,
                        op=mybir.AluOpType.min)
                    f = ph3sm.tile([128, NG_K], F32, tag="f")
                    nc.scalar.activation(
                        f[:], mloc[:], mybir.ActivationFunctionType.Exp,
                        bias=negm[:], scale=-1.0)
                    fl = ph3sm.tile([128, NG_K], F32, tag="fl")
                    nc.vector.tensor_tensor(
                        fl[:], f[:], lparts[:], op=mybir.AluOpType.mult)
                    l = ph3sm.tile([128, 1], F32, tag="l")
                    nc.vector.tensor_reduce(
                        l[:], fl[:], axis=mybir.AxisListType.X,
                        op=mybir.AluOpType.add)
                    linv = ph3sm.tile([128, 1], F32, tag="linv")
                    nc.vector.reciprocal(linv[:], l[:])
                    for g in range(NG_K):
                        eng = nc.gpsimd if g < npool else nc.vector
                        eng.tensor_scalar_mul(
                            w[:, g * GK:(g + 1) * GK],
                            w[:, g * GK:(g + 1) * GK], f[:, g:g + 1])
                    wTt = ph3wt.tile([128, LKV // 128, 128], FP16, tag="wTt")
                    if t % NT_Q < n_pe_transp:
                        for qq in range(8):
                            pt = psB.tile([128, 4, 128], FP16, tag="pt")
                            for s in range(4):
                                i = qq * 4 + s
                                nc.tensor.transpose(
                                    pt[:, s, :],
                                    w[:, i * 128:(i + 1) * 128], identw[:])
                            if qq % 2 == 0:
                                nc.scalar.copy(wTt[:, 4 * qq:4 * (qq + 1), :], pt[:])
                            else:
                                nc.vector.tensor_copy(
                                    wTt[:, 4 * qq:4 * (qq + 1), :], pt[:])
                    else:
                        nc.sync.dma_start_transpose(wTt[:], w[:])
                    ops = psB.tile([128, 4, 128], F32, tag="vps")
                    for i in range(LKV // 128):
                        nc.tensor.matmul(
                            ops[:, 0, :], wTt[:, i, :],
                            v[:, i * 128:(i + 1) * 128],
                            start=(i == 0), stop=(i == LKV // 128 - 1))
                    osb = ph3o.tile([128, 128], F32, tag="osb")
                    nc.scalar.activation(osb[:], ops[:, 0, :],
                                         mybir.ActivationFunctionType.Copy,
                                         scale=linv[:])
                    nc.sync.dma_start(out[tq, :], osb[:])
    nc.finalize()
    return nc


_NC_CACHE = None
TRACE = False
LAST_EXEC_NS = None
LAST_RESULTS = None


def _split16_8(a):
    """a (fp32) -> (fp16 hi, e5m2 of hi, e5m2 of lo)"""
    import ml_dtypes
    hi = a.astype(np.float16)
    lo8 = (a - hi.astype(np.float32)).astype(ml_dtypes.float8_e5m2)
    hi8 = hi.astype(ml_dtypes.float8_e5m2)
    return hi, hi8, lo8


def kernel(x, z, Wq, Wk, Wv):
    global _NC_CACHE, LAST_EXEC_NS, LAST_RESULTS
    if _NC_CACHE is None:
        _NC_CACHE = build_bass()
    nc = _NC_CACHE

    x = np.asarray(x, dtype=np.float32)
    z = np.asarray(z, dtype=np.float32)
    Wqs = np.ascontiguousarray(np.asarray(Wq, dtype=np.float32).T) * WS
    Wks = np.ascontiguousarray(np.asarray(Wk, dtype=np.float32).T) * WS
    WvT = np.ascontiguousarray(np.asarray(Wv, dtype=np.float32).T)

    Wqh, Wqh8, Wql8 = _split16_8(Wqs)
    Wkh, Wkh8, Wkl8 = _split16_8(Wks)
    # W planes: (lo8, hi8) pair with input planes (hi8, lo8)
    Wq8 = np.ascontiguousarray(np.stack([Wql8, Wqh8], axis=1))
    Wk8 = np.ascontiguousarray(np.stack([Wkl8, Wkh8], axis=1))
    Wvh = WvT.astype(np.float16)

    in_maps = []
    for core in range(N_CORES):
        b, half = core // 2, core % 2
        rows = slice(half * LQS, (half + 1) * LQS)
        xT = np.ascontiguousarray(x[b, rows].T)
        zT = np.ascontiguousarray(z[b].T)
        xhh, xh8, xl8 = _split16_8(xT)
        zhh, zh8, zl8 = _split16_8(zT)
        in_maps.append({
            "xh": xhh, "x8": np.ascontiguousarray(np.stack([xh8, xl8], axis=1)),
            "zh": zhh, "z8": np.ascontiguousarray(np.stack([zh8, zl8], axis=1)),
            "Wqh": Wqh, "Wq8": Wq8, "Wkh": Wkh, "Wk8": Wk8, "Wvh": Wvh,
        })
    if TRACE:
        tdir = "/root/problem/trace_out"
        os.makedirs(tdir, exist_ok=True)
        br = run_bass_kernel_spmd(nc, in_maps, list(range(N_CORES)),
                                  trace=True, tmpdir=tdir)
        LAST_EXEC_NS = br.exec_time_ns
        LAST_RESULTS = br
        res = br.results
    else:
        res = run_bass_kernel_spmd(nc, in_maps, list(range(N_CORES))).results
    outp = np.empty((B, LQ, H), dtype=np.float32)
    for core in range(N_CORES):
        b, half = core // 2, core % 2
        outp[b, half * LQS:(half + 1) * LQS] = res[core]["out"]
    return outp
